# revision 1
# baseline (speedup 1.0000x reference)
"""DeepSeek-style MoE (top-2 of 16 routed experts + 2 dense shared experts)
on 8 Trainium2 NeuronCores.

Sharding (hardcoded for x:[4,2048,2048], D=2048, E=16, H_R=512, H_S=8192):
  - Gate (softmax + top-2) is computed on host as part of the dispatch step,
    then tokens are packed per expert (all-to-all done host-side while
    building the per-core shards).
  - Shared experts: data-parallel, 1024 tokens per core, full shared weights
    replicated per core and streamed through SBUF exactly once.
  - Routed experts: expert-parallel, 2 experts per core, capacity 1280
    token-slots per expert (avg load is 1024, observed max 1087); combine
    weights applied on-device; slots beyond capacity fall back to an exact
    host computation.

Device kernel (single SPMD program on all 8 cores, bf16 matmuls with fp32
PSUM accumulation):
  - activations are provided D-major (host pre-transposes once), weights are
    used in their natural [in,out] layout as the stationary operand, and the
    second FFN layer swaps matmul operands (lhsT = hidden tile) so outputs
    come out token-major -> no transposes on device and no output transposes
    on host.
  - Layer pair is fused through SBUF with H-chunking (chunk=512): hidden
    activations never touch DRAM; second-layer partial products accumulate
    into resident fp32 SBUF tiles via DVE adds. Weights stream through SBUF
    exactly once per core (~200 MB vs ~2 ms of PE work -> compute-bound).
  - Measured: ~1.97 ms HW time/core, PE matmul occupancy ~98%, rel err 3.4e-3.
"""
import sys
import types
from contextlib import ExitStack

import numpy as np

_TRN = "/opt/trn_rl_repo"
if _TRN not in sys.path:
    sys.path.insert(0, _TRN)

import ml_dtypes  # noqa: E402
import concourse.mybir as mybir  # noqa: E402
import concourse.tile as tile  # noqa: E402
from concourse import bacc  # noqa: E402
from concourse.bass_utils import run_bass_kernel_spmd  # noqa: E402

BF16 = mybir.dt.bfloat16
F32 = mybir.dt.float32
GELU = mybir.ActivationFunctionType.Gelu
ADD = mybir.AluOpType.add
MULT = mybir.AluOpType.mult
bf16_np = ml_dtypes.bfloat16

P = 128
D = 2048          # model dim
E = 16            # routed experts
TOPK = 2
HS = 8192         # shared-expert hidden
HR = 512          # routed-expert hidden
S_EXP = 2         # shared experts
NCORES = 8
N = 8192          # tokens
TPC = N // NCORES     # tokens per core (1024)
EPC = E // NCORES     # routed experts per core (2)
CAP = 1280            # routed capacity per expert (avg load 1024, max seen ~1090)
HALF = 256            # routed tokens processed per pass
NKD = D // P          # 16 contraction tiles over D
CH = 512              # shared-expert H chunk
NCH = HS // CH        # 16 chunks per shared expert
NT = TPC // P         # 8 token tiles per core
NDC = D // 512        # 4 output-D chunks


def _emit(nc, tc, ctx, t):
    """Emit the tile program. `t` is the dict of DRAM tensor handles."""
    xacts = ctx.enter_context(tc.tile_pool(name="xacts", bufs=16))
    wslab = ctx.enter_context(tc.tile_pool(name="wslab", bufs=32))
    xepool = ctx.enter_context(tc.tile_pool(name="xepool", bufs=32))
    w2slab = ctx.enter_context(tc.tile_pool(name="w2slab", bufs=6))
    hpool = ctx.enter_context(tc.tile_pool(name="hpool", bufs=10))
    ypool = ctx.enter_context(tc.tile_pool(name="ypool", bufs=8))
    cpool = ctx.enter_context(tc.tile_pool(name="cpool", bufs=1))
    psA = ctx.enter_context(tc.tile_pool(name="psA", bufs=2, space="PSUM"))
    psB = ctx.enter_context(tc.tile_pool(name="psB", bufs=3, space="PSUM"))

    # constants
    sb1T = cpool.tile([P, S_EXP * HS // P], F32, name="sb1T")       # [128, 128]
    nc.sync.dma_start(sb1T[:], t["sb1T"][:, :])
    eb1T = cpool.tile([P, EPC * HR // P], F32, name="eb1T")         # [128, 8]
    nc.sync.dma_start(eb1T[:], t["eb1T"][:, :])

    # x^T resident: 16 tiles [128, 1024] bf16 (host provides x pre-transposed).
    # Interleave with chunk-0 W1 slab loads so the first psum group's deps
    # complete as early as possible.
    xT = []
    w1s_first = []
    for k in range(NKD):
        xt = xacts.tile([P, TPC], BF16, name="xT", tag="xacts")
        nc.sync.dma_start(xt[:], t["xT_tok"][k * P:(k + 1) * P, :])
        xT.append(xt)
        w = wslab.tile([P, CH], BF16, name="w1s", tag="wslab")
        nc.sync.dma_start(w[:], t["sw1"][0, k * P:(k + 1) * P, 0:CH])
        w1s_first.append(w)

    y_tiles = [None] * NT

    # ---- shared experts: y[tok, D] += sum_s W2_s^T gelu(W1_s^T x^T + b1) ----
    for s in range(S_EXP):
        for c in range(NCH):
            first = (s == 0 and c == 0)
            # phase A: hT chunk [CH, TPC] = gelu(W1[:, chunk]^T @ xT + b1)
            if first:
                w1s = w1s_first
            else:
                w1s = []
                for k in range(NKD):
                    w = wslab.tile([P, CH], BF16, name="w1s", tag="wslab")
                    nc.sync.dma_start(
                        w[:],
                        t["sw1"][s, k * P:(k + 1) * P, c * CH:(c + 1) * CH])
                    w1s.append(w)
            hts = []
            for h in range(CH // P):
                ps = psA.tile([P, TPC], F32, name="psA", tag="psA")
                for k in range(NKD):
                    for n in range(TPC // 512):
                        nc.tensor.matmul(
                            ps[:, n * 512:(n + 1) * 512],
                            w1s[k][:, h * P:(h + 1) * P],
                            xT[k][:, n * 512:(n + 1) * 512],
                            start=(k == 0), stop=(k == NKD - 1))
                ht = hpool.tile([P, TPC], BF16, name="ht", tag="hpool")
                nc.scalar.activation(ht[:], ps[:], GELU,
                                     bias=sb1T[:, s * 64 + c * 4 + h:
                                               s * 64 + c * 4 + h + 1])
                hts.append(ht)
            # phase B: y[tok, :] += W2[chunk, :]^T-contracted, token-major via
            # swapped operands: out = hT_tile.T @ w2_slab
            w2s = []
            for kh in range(CH // P):
                w = w2slab.tile([P, D], BF16, name="w2s", tag="w2slab")
                nc.sync.dma_start(
                    w[:], t["sw2"][s, c * CH + kh * P:c * CH + (kh + 1) * P, :])
                w2s.append(w)
            for ti in range(NT):
                for n in range(NDC):
                    ps = psB.tile([P, 512], F32, name="psB", tag="psB")
                    for kh in range(CH // P):
                        nc.tensor.matmul(
                            ps[:, :],
                            hts[kh][:, ti * P:(ti + 1) * P],
                            w2s[kh][:, n * 512:(n + 1) * 512],
                            start=(kh == 0), stop=(kh == CH // P - 1))
                    if first:
                        if n == 0:
                            y_tiles[ti] = ypool.tile([P, D], F32, name="y",
                                                     tag="ypool")
                        nc.vector.tensor_copy(
                            y_tiles[ti][:, n * 512:(n + 1) * 512], ps[:, :])
                    else:
                        nc.vector.tensor_tensor(
                            y_tiles[ti][:, n * 512:(n + 1) * 512],
                            y_tiles[ti][:, n * 512:(n + 1) * 512],
                            ps[:, :], op=ADD)

    for ti in range(NT):
        nc.sync.dma_start(t["ysh"][ti * P:(ti + 1) * P, :], y_tiles[ti][:])

    # ---- routed experts (2 per core, CAP slots each, passes of HALF) ----
    # Software-pipelined: A(pass p+1) is emitted before B(pass p) so the PE
    # fills the gelu/scale latency window of pass p with pass p+1's matmuls.
    expert_e1s = {}
    expert_e2s = {}

    def load_e1s(e):
        e1s = []
        for k in range(NKD):
            w = wslab.tile([P, HR], BF16, name="e1s", tag="wslab")
            nc.sync.dma_start(w[:], t["ew1"][e, k * P:(k + 1) * P, :])
            e1s.append(w)
        expert_e1s[e] = e1s

    def load_e2s(e):
        e2s = []
        for kh in range(HR // P):
            w = w2slab.tile([P, D], BF16, name="e2s", tag="w2slab")
            nc.sync.dma_start(w[:], t["ew2"][e, kh * P:(kh + 1) * P, :])
            e2s.append(w)
        expert_e2s[e] = e2s

    def emit_A(e, half):
        e1s = expert_e1s[e]
        sc = hpool.tile([P, HALF], BF16, name="sc", tag="hpool")
        nc.sync.dma_start(
            sc[:], t["scaleb"][e, :, half * HALF:(half + 1) * HALF])
        xe = []
        for k in range(NKD):
            xt = xepool.tile([P, HALF], BF16, name="xe", tag="xepool")
            nc.sync.dma_start(
                xt[:], t["xeT_tok"][e, k * P:(k + 1) * P,
                                    half * HALF:(half + 1) * HALF])
            xe.append(xt)
        hts = []
        for h in range(HR // P):
            ps = psA.tile([P, HALF], F32, name="psAr", tag="psA")
            for k in range(NKD):
                nc.tensor.matmul(
                    ps[:, :],
                    e1s[k][:, h * P:(h + 1) * P],
                    xe[k][:, :],
                    start=(k == 0), stop=(k == NKD - 1))
            ht = hpool.tile([P, HALF], BF16, name="htr", tag="hpool")
            nc.scalar.activation(ht[:], ps[:], GELU,
                                 bias=eb1T[:, e * 4 + h:e * 4 + h + 1])
            nc.vector.tensor_tensor(ht[:], ht[:], sc[:], op=MULT)
            hts.append(ht)
        return hts

    def emit_B(e, half, hts):
        e2s = expert_e2s[e]
        for ti in range(HALF // P):
            st = ypool.tile([P, D], F32, name="str", tag="ypool")
            for n in range(NDC):
                ps = psB.tile([P, 512], F32, name="psBr", tag="psB")
                for kh in range(HR // P):
                    nc.tensor.matmul(
                        ps[:, :],
                        hts[kh][:, ti * P:(ti + 1) * P],
                        e2s[kh][:, n * 512:(n + 1) * 512],
                        start=(kh == 0), stop=(kh == HR // P - 1))
                nc.vector.tensor_copy(st[:, n * 512:(n + 1) * 512], ps[:, :])
            nc.sync.dma_start(
                t["yrt"][e, half * HALF + ti * P:
                         half * HALF + (ti + 1) * P, :], st[:])

    NPASS = CAP // HALF
    load_e1s(0)
    load_e2s(0)
    pending = None
    for e in range(EPC):
        for half in range(NPASS):
            if e + 1 < EPC and half == max(NPASS - 2, 1):
                # prefetch next expert's first-layer slabs two passes early
                load_e1s(e + 1)
            hts = emit_A(e, half)
            if pending is not None:
                emit_B(*pending)
            if e + 1 < EPC and half == NPASS - 1:
                load_e2s(e + 1)
            pending = (e, half, hts)
    emit_B(*pending)


def _install_neff_cache():
    """Disk-cache walrus NEFF compiles keyed by BIR hash (compile is ~5min)."""
    import concourse.bass2jax as b2j
    if getattr(b2j, "_neff_cache_installed", False):
        return
    import hashlib
    import os
    import shutil
    orig = b2j.compile_bir_kernel
    cache_dir = "/tmp/bass_neff_cache"

    def cached(bir_json, tmpdir, neff_name="file.neff"):
        try:
            os.makedirs(cache_dir, exist_ok=True)
            h = hashlib.sha256(bir_json).hexdigest()[:24]
            cpath = os.path.join(cache_dir, h + ".neff")
            if os.path.exists(cpath):
                dst = os.path.join(tmpdir, neff_name)
                shutil.copy(cpath, dst)
                return dst
            p = orig(bir_json, tmpdir, neff_name)
            shutil.copy(p, cpath)
            return p
        except OSError:
            return orig(bir_json, tmpdir, neff_name)

    b2j.compile_bir_kernel = cached
    b2j._neff_cache_installed = True


_CACHE = {}


def _get_compiled():
    if "nc" in _CACHE:
        return _CACHE["nc"]
    nc = bacc.Bacc("TRN2", target_bir_lowering=False, debug=False,
                   num_devices=NCORES)
    t = {}
    t["xT_tok"] = nc.dram_tensor("xT_tok", [D, TPC], BF16,
                                 kind="ExternalInput")
    t["xeT_tok"] = nc.dram_tensor("xeT_tok", [EPC, D, CAP], BF16,
                                  kind="ExternalInput")
    t["sw1"] = nc.dram_tensor("sw1", [S_EXP, D, HS], BF16, kind="ExternalInput")
    t["sw2"] = nc.dram_tensor("sw2", [S_EXP, HS, D], BF16, kind="ExternalInput")
    t["ew1"] = nc.dram_tensor("ew1", [EPC, D, HR], BF16, kind="ExternalInput")
    t["ew2"] = nc.dram_tensor("ew2", [EPC, HR, D], BF16, kind="ExternalInput")
    t["scaleb"] = nc.dram_tensor("scaleb", [EPC, P, CAP], BF16,
                                 kind="ExternalInput")
    t["sb1T"] = nc.dram_tensor("sb1T", [P, S_EXP * HS // P], F32,
                               kind="ExternalInput")
    t["eb1T"] = nc.dram_tensor("eb1T", [P, EPC * HR // P], F32,
                               kind="ExternalInput")
    t["ysh"] = nc.dram_tensor("ysh", [TPC, D], F32, kind="ExternalOutput")
    t["yrt"] = nc.dram_tensor("yrt", [EPC, CAP, D], F32, kind="ExternalOutput")

    with tile.TileContext(nc) as tc, ExitStack() as ctx:
        _emit(nc, tc, ctx, t)
    nc.compile()
    _CACHE["nc"] = nc
    return nc


def _install_profile_hook():
    """Make run_bass_kernel_spmd(trace=True) work in this image (the antenv
    package lacks axon_hooks; provide it and register the ctypes hook)."""
    try:
        from antenv import axon_hooks  # noqa: F401
        return
    except ImportError:
        pass
    import antenv
    mod = types.ModuleType("antenv.axon_hooks")
    _hook = [None]
    mod.set_axon_ntff_profile_hook = lambda h: _hook.__setitem__(0, h)
    mod.get_axon_ntff_profile_hook = lambda: _hook[0]
    sys.modules["antenv.axon_hooks"] = mod
    antenv.axon_hooks = mod
    try:
        from trn_agent_boot.trn_boot import _ntff_profile_via_ctypes
        hook = _ntff_profile_via_ctypes("/opt/axon/libaxon_pjrt.so")
        if hook is not None:
            mod.set_axon_ntff_profile_hook(hook)
    except Exception:
        pass


def _gelu_np(x):
    from scipy.special import erf
    return 0.5 * x * (1.0 + erf(x / np.sqrt(2.0)))


def kernel(x, gate_w, gate_b, ew1, eb1, ew2, eb2, sw1, sb1, sw2, sb2,
           _trace=False, _trace_cores=None):
    x = np.asarray(x, np.float32)
    gate_w = np.asarray(gate_w, np.float32)
    gate_b = np.asarray(gate_b, np.float32)
    ew1 = np.asarray(ew1, np.float32)
    eb1 = np.asarray(eb1, np.float32)
    ew2 = np.asarray(ew2, np.float32)
    eb2 = np.asarray(eb2, np.float32)
    sw1 = np.asarray(sw1, np.float32)
    sb1 = np.asarray(sb1, np.float32)
    sw2 = np.asarray(sw2, np.float32)
    sb2 = np.asarray(sb2, np.float32)

    b, s, d = x.shape
    assert b * s == N and d == D, (x.shape, "kernel hardcodes [4,2048,2048]")
    xf = np.ascontiguousarray(x.reshape(-1, d))

    # ---- routing on host (this *is* the dispatch/sharding step) ----
    logits = xf @ gate_w + gate_b
    logits -= logits.max(axis=-1, keepdims=True)
    g = np.exp(logits, dtype=np.float32)
    g /= g.sum(axis=-1, keepdims=True)
    topi = np.argpartition(-g, TOPK, axis=1)[:, :TOPK]          # [N, 2]
    topv = np.take_along_axis(g, topi, axis=1)                  # [N, 2]

    flat_e = topi.ravel()                                       # pair p = 2n+k
    flat_w = topv.ravel()
    flat_tok = np.repeat(np.arange(N, dtype=np.int64), TOPK)
    order = np.argsort(flat_e, kind="stable")
    counts = np.bincount(flat_e, minlength=E)
    starts = np.concatenate([[0], np.cumsum(counts)[:-1]])
    ranks = np.empty(N * TOPK, np.int64)
    ranks[order] = np.arange(N * TOPK) - starts[flat_e[order]]
    ok = ranks < CAP

    # pack tokens per expert (bf16, D-major), padding slots -> zero column
    xfb = xf.astype(bf16_np)
    xT_all = np.ascontiguousarray(xfb.T)                        # [D, N]
    xT_pad = np.concatenate([xT_all, np.zeros((D, 1), bf16_np)], axis=1)
    xe_idx = np.full((E, CAP), N, np.int64)
    xe_idx[flat_e[ok], ranks[ok]] = flat_tok[ok]
    xeT_all = xT_pad[:, xe_idx.reshape(-1)].reshape(D, E, CAP)  # [D, E, CAP]

    sc_all = np.zeros((E, CAP), np.float32)
    sc_all[flat_e[ok], ranks[ok]] = flat_w[ok]
    sc_b = np.ascontiguousarray(
        np.broadcast_to(sc_all[:, None, :], (E, P, CAP))).astype(bf16_np)

    sw1b = sw1.astype(bf16_np)
    sw2b = sw2.astype(bf16_np)
    ew1b = ew1.astype(bf16_np)
    ew2b = ew2.astype(bf16_np)
    sb1T = np.ascontiguousarray(
        sb1.reshape(S_EXP * HS // P, P).T).astype(np.float32)
    sb2_sum = sb2.sum(axis=0).astype(np.float32)

    _install_neff_cache()
    nc = _get_compiled()
    if _trace:
        _install_profile_hook()

    in_maps = []
    for c in range(NCORES):
        el, eh = c * EPC, (c + 1) * EPC
        eb1T = np.ascontiguousarray(
            eb1[el:eh].reshape(EPC * HR // P, P).T).astype(np.float32)
        in_maps.append({
            "xT_tok": np.ascontiguousarray(xT_all[:, c * TPC:(c + 1) * TPC]),
            "xeT_tok": np.ascontiguousarray(
                xeT_all[:, el:eh, :].transpose(1, 0, 2)),
            "sw1": sw1b,
            "sw2": sw2b,
            "ew1": np.ascontiguousarray(ew1b[el:eh]),
            "ew2": np.ascontiguousarray(ew2b[el:eh]),
            "scaleb": np.ascontiguousarray(sc_b[el:eh]),
            "sb1T": sb1T,
            "eb1T": eb1T,
        })

    if _trace and _trace_cores is None:
        _trace_cores = list(range(NCORES))
    res = run_bass_kernel_spmd(
        nc, in_maps, core_ids=list(range(NCORES)),
        trace=_trace, trace_cores=_trace_cores if _trace else None)
    kernel.last_results = res

    # ---- assemble ----
    out = np.empty((N, D), np.float32)
    for c in range(NCORES):
        out[c * TPC:(c + 1) * TPC] = res.results[c]["ysh"] + sb2_sum

    yrt_all = np.empty((E, CAP, D), np.float32)
    for c in range(NCORES):
        yrt_all[c * EPC:(c + 1) * EPC] = res.results[c]["yrt"]
    flat_rows = yrt_all.reshape(E * CAP, D)
    for k in range(TOPK):
        pk = np.arange(N) * TOPK + k
        okk = ok[pk]
        pos = flat_e[pk] * CAP + ranks[pk]
        if okk.all():
            out += flat_rows[pos]
        else:
            out[okk] += flat_rows[pos[okk]]
            # exact host fallback for overflow assignments, batched per expert
            bad = np.nonzero(~okk)[0]
            for e_ in np.unique(flat_e[pk[bad]]):
                sel = bad[flat_e[pk[bad]] == e_]
                h_ = _gelu_np(xf[sel] @ ew1[e_] + eb1[e_])
                out[sel] += flat_w[pk[sel], None] * (h_ @ ew2[e_] + eb2[e_])

    if np.any(eb2):
        for k in range(TOPK):
            out += topv[:, k:k + 1] * eb2[topi[:, k]]

    return out.reshape(b, s, d)



# revision 3
# speedup vs baseline: 1.2349x; 1.2349x over previous
"""DeepSeek-style MoE (top-2 of 16 routed experts + 2 dense shared experts)
on 8 Trainium2 NeuronCores.

Sharding (hardcoded for x:[4,2048,2048], D=2048, E=16, H_R=512, H_S=8192):
  - Gate (softmax + top-2) on host as part of dispatch; tokens packed per
    expert host-side while building per-core shards.
  - Shared experts: data-parallel, 1024 tokens/core, bf16 matmuls, weights
    streamed through SBUF once. L1->L2 software-pipelined ACROSS chunks so
    the PE never waits on the gelu at a chunk boundary.
  - Routed experts: expert-parallel, 2 experts/core, capacity 1152 slots
    (max observed load 1087), computed ENTIRELY in fp8(e4m3) with
    MatmulPerfMode.DoubleRow (2x bf16 MAC rate; contributes ~0.3% rel err
    because routed RMS is ~20x below shared RMS). Weights carry a pow2
    scale (folded out via the gelu activation scale on L1 and a host-side
    descale of yrt for L2). Combine weights applied on-device; overflow
    slots fall back to exact host computation.
"""
import sys
import types
from contextlib import ExitStack

import numpy as np

_TRN = "/opt/trn_rl_repo"
if _TRN not in sys.path:
    sys.path.insert(0, _TRN)

import ml_dtypes  # noqa: E402
import concourse.mybir as mybir  # noqa: E402
import concourse.tile as tile  # noqa: E402
from concourse import bacc  # noqa: E402
from concourse.bass_utils import run_bass_kernel_spmd  # noqa: E402

BF16 = mybir.dt.bfloat16
F8 = mybir.dt.float8e4
F32 = mybir.dt.float32
GELU = mybir.ActivationFunctionType.Gelu
ADD = mybir.AluOpType.add
MULT = mybir.AluOpType.mult
DR = mybir.MatmulPerfMode.DoubleRow
bf16_np = ml_dtypes.bfloat16
f8_np = ml_dtypes.float8_e4m3

P = 128
D = 2048          # model dim
E = 16            # routed experts
TOPK = 2
HS = 8192         # shared-expert hidden
HR = 512          # routed-expert hidden
S_EXP = 2         # shared experts
NCORES = 8
N = 8192          # tokens
TPC = N // NCORES     # tokens per core (1024)
EPC = E // NCORES     # routed experts per core (2)
CAP = 1152            # routed capacity per expert (max seen load 1087)
HALF = 384            # routed tokens per pass
NPASS = CAP // HALF   # 3
NKD = D // P          # 16 contraction tiles over D (bf16 shared path)
NKP = D // (2 * P)    # 8 k-tile PAIRS over D (fp8 DoubleRow routed path)
RKP = HR // (2 * P)   # 2 hidden k-tile pairs (routed L2)
CH = 512              # shared-expert H chunk
NCH = HS // CH        # 16 chunks per shared expert
NT = TPC // P         # 8 token tiles per core
NDC = D // 512        # 4 output-D chunks

S_E1 = 1024.0         # pow2 scale baked into routed fp8 weights (W*S fits e4m3)
S_E2 = 1024.0


def _emit(nc, tc, ctx, t):
    """Emit the tile program. `t` is the dict of DRAM tensor handles."""
    xacts = ctx.enter_context(tc.tile_pool(name="xacts", bufs=16))
    wslab = ctx.enter_context(tc.tile_pool(name="wslab", bufs=32))
    xepool = ctx.enter_context(tc.tile_pool(name="xepool", bufs=16))
    w2slab = ctx.enter_context(tc.tile_pool(name="w2slab", bufs=6))
    hpool = ctx.enter_context(tc.tile_pool(name="hpool", bufs=12))
    hqpool = ctx.enter_context(tc.tile_pool(name="hqpool", bufs=6))
    ypool = ctx.enter_context(tc.tile_pool(name="ypool", bufs=8))
    cpool = ctx.enter_context(tc.tile_pool(name="cpool", bufs=1))
    psA = ctx.enter_context(tc.tile_pool(name="psA", bufs=2, space="PSUM"))
    psB = ctx.enter_context(tc.tile_pool(name="psB", bufs=3, space="PSUM"))

    # constants
    sb1T = cpool.tile([P, S_EXP * HS // P], F32, name="sb1T")       # [128, 128]
    nc.sync.dma_start(sb1T[:], t["sb1T"][:, :])
    eb1T = cpool.tile([P, EPC * HR // P], F32, name="eb1T")         # [128, 8]
    nc.sync.dma_start(eb1T[:], t["eb1T"][:, :])

    # x^T resident: 16 tiles [128, 1024] bf16 (host provides x pre-transposed).
    xT = []
    w1s_first = []
    for k in range(NKD):
        xt = xacts.tile([P, TPC], BF16, name="xT", tag="xacts")
        nc.sync.dma_start(xt[:], t["xT_tok"][k * P:(k + 1) * P, :])
        xT.append(xt)
        w = wslab.tile([P, CH], BF16, name="w1s", tag="wslab")
        nc.sync.dma_start(w[:], t["sw1"][0, k * P:(k + 1) * P, 0:CH])
        w1s_first.append(w)

    y_tiles = [None] * NT

    # ---- shared experts: y[tok, D] += sum_s W2_s^T gelu(W1_s^T x^T + b1) ----
    # A(s,c): hT chunk [CH, TPC] = gelu(W1[:, chunk]^T @ xT + b1)
    # B(s,c): y += hT-contracted W2 chunk.  Emitted pipelined: A(next) before
    # B(cur) so the PE fills the gelu latency window with A's matmuls.
    def emit_sh_A(s, c):
        if s == 0 and c == 0:
            w1s = w1s_first
        else:
            w1s = []
            for k in range(NKD):
                w = wslab.tile([P, CH], BF16, name="w1s", tag="wslab")
                nc.sync.dma_start(
                    w[:],
                    t["sw1"][s, k * P:(k + 1) * P, c * CH:(c + 1) * CH])
                w1s.append(w)
        hts = []
        for h in range(CH // P):
            ps = psA.tile([P, TPC], F32, name="psA", tag="psA")
            for k in range(NKD):
                for n in range(TPC // 512):
                    nc.tensor.matmul(
                        ps[:, n * 512:(n + 1) * 512],
                        w1s[k][:, h * P:(h + 1) * P],
                        xT[k][:, n * 512:(n + 1) * 512],
                        start=(k == 0), stop=(k == NKD - 1))
            ht = hpool.tile([P, TPC], BF16, name="ht", tag="hpool")
            nc.scalar.activation(ht[:], ps[:], GELU,
                                 bias=sb1T[:, s * 64 + c * 4 + h:
                                           s * 64 + c * 4 + h + 1])
            hts.append(ht)
        return hts

    def emit_sh_B(s, c, hts):
        first = (s == 0 and c == 0)
        w2s = []
        for kh in range(CH // P):
            w = w2slab.tile([P, D], BF16, name="w2s", tag="w2slab")
            nc.sync.dma_start(
                w[:], t["sw2"][s, c * CH + kh * P:c * CH + (kh + 1) * P, :])
            w2s.append(w)
        for ti in range(NT):
            for n in range(NDC):
                ps = psB.tile([P, 512], F32, name="psB", tag="psB")
                for kh in range(CH // P):
                    nc.tensor.matmul(
                        ps[:, :],
                        hts[kh][:, ti * P:(ti + 1) * P],
                        w2s[kh][:, n * 512:(n + 1) * 512],
                        start=(kh == 0), stop=(kh == CH // P - 1))
                if first:
                    if n == 0:
                        y_tiles[ti] = ypool.tile([P, D], F32, name="y",
                                                 tag="ypool")
                    nc.vector.tensor_copy(
                        y_tiles[ti][:, n * 512:(n + 1) * 512], ps[:, :])
                else:
                    nc.vector.tensor_tensor(
                        y_tiles[ti][:, n * 512:(n + 1) * 512],
                        y_tiles[ti][:, n * 512:(n + 1) * 512],
                        ps[:, :], op=ADD)

    sh_pending = None
    for s in range(S_EXP):
        for c in range(NCH):
            hts = emit_sh_A(s, c)
            if sh_pending is not None:
                emit_sh_B(*sh_pending)
            sh_pending = (s, c, hts)
    # last shared B is emitted interleaved with the first routed A below.

    # ---- routed experts: fp8 DoubleRow, 2 experts/core, NPASS passes ----
    expert_e1s = {}
    expert_e2s = {}

    def load_e1s(e):
        e1s = []
        for kp in range(NKP):
            w = wslab.tile([P, 2, HR], F8, name="e1s", tag="wslab")
            nc.sync.dma_start(w[:], t["ew1q"][e, kp, :, :, :])
            e1s.append(w)
        expert_e1s[e] = e1s

    def load_e2s(e):
        e2s = []
        for kp in range(RKP):
            w = w2slab.tile([P, 2, D], F8, name="e2s", tag="w2slab")
            nc.sync.dma_start(w[:], t["ew2q"][e, kp, :, :, :])
            e2s.append(w)
        expert_e2s[e] = e2s

    def emit_A(e, half):
        e1s = expert_e1s[e]
        sc = hpool.tile([P, HALF], BF16, name="sc", tag="hpool")
        nc.sync.dma_start(sc[:], t["scaleb"][e, half, :, :])
        xe = []
        for kp in range(NKP):
            xt = xepool.tile([P, 2, HALF], F8, name="xe", tag="xepool")
            nc.sync.dma_start(xt[:], t["xe8"][e, kp, half, :, :, :])
            xe.append(xt)
        hqs = [hqpool.tile([P, 2, HALF], F8, name="hq", tag="hqpool")
               for _ in range(RKP)]
        for h in range(HR // P):
            ps = psA.tile([P, HALF], F32, name="psAr", tag="psA")
            for kp in range(NKP):
                nc.tensor.matmul(
                    ps[:, :],
                    e1s[kp][:, :, h * P:(h + 1) * P],
                    xe[kp][:, :, :],
                    start=(kp == 0), stop=(kp == NKP - 1),
                    perf_mode=DR)
            ht = hpool.tile([P, HALF], BF16, name="htr", tag="hpool")
            nc.scalar.activation(ht[:], ps[:], GELU,
                                 bias=eb1T[:, e * 4 + h:e * 4 + h + 1],
                                 scale=1.0 / S_E1)
            nc.vector.tensor_tensor(hqs[h // 2][:, h % 2, :],
                                    ht[:], sc[:], op=MULT)
        return hqs

    def emit_B(e, half, hqs):
        e2s = expert_e2s[e]
        for ti in range(HALF // P):
            st = ypool.tile([P, D], F32, name="str", tag="ypool")
            for n in range(NDC):
                ps = psB.tile([P, 512], F32, name="psBr", tag="psB")
                for kp in range(RKP):
                    nc.tensor.matmul(
                        ps[:, :],
                        hqs[kp][:, :, ti * P:(ti + 1) * P],
                        e2s[kp][:, :, n * 512:(n + 1) * 512],
                        start=(kp == 0), stop=(kp == RKP - 1),
                        perf_mode=DR)
                nc.vector.tensor_copy(st[:, n * 512:(n + 1) * 512], ps[:, :])
            nc.sync.dma_start(
                t["yrt"][e, half * HALF + ti * P:
                         half * HALF + (ti + 1) * P, :], st[:])

    load_e1s(0)
    load_e2s(0)
    pending = None
    for e in range(EPC):
        for half in range(NPASS):
            if e + 1 < EPC and half == max(NPASS - 2, 1):
                load_e1s(e + 1)
            hqs = emit_A(e, half)
            if sh_pending is not None:
                # last shared-expert B slots in behind the first routed A
                emit_sh_B(*sh_pending)
                sh_pending = None
                for ti in range(NT):
                    nc.sync.dma_start(t["ysh"][ti * P:(ti + 1) * P, :],
                                      y_tiles[ti][:])
            if pending is not None:
                emit_B(*pending)
            if e + 1 < EPC and half == NPASS - 1:
                load_e2s(e + 1)
            pending = (e, half, hqs)
    emit_B(*pending)


def _install_neff_cache():
    """Disk-cache walrus NEFF compiles keyed by BIR hash (compile is ~5min)."""
    import concourse.bass2jax as b2j
    if getattr(b2j, "_neff_cache_installed", False):
        return
    import hashlib
    import os
    import shutil
    orig = b2j.compile_bir_kernel
    cache_dir = "/tmp/bass_neff_cache"

    def cached(bir_json, tmpdir, neff_name="file.neff"):
        try:
            os.makedirs(cache_dir, exist_ok=True)
            h = hashlib.sha256(bir_json).hexdigest()[:24]
            cpath = os.path.join(cache_dir, h + ".neff")
            if os.path.exists(cpath):
                dst = os.path.join(tmpdir, neff_name)
                shutil.copy(cpath, dst)
                return dst
            p = orig(bir_json, tmpdir, neff_name)
            shutil.copy(p, cpath)
            return p
        except OSError:
            return orig(bir_json, tmpdir, neff_name)

    b2j.compile_bir_kernel = cached
    b2j._neff_cache_installed = True


_CACHE = {}


def _get_compiled():
    if "nc" in _CACHE:
        return _CACHE["nc"]
    nc = bacc.Bacc("TRN2", target_bir_lowering=False, debug=False,
                   num_devices=NCORES)
    t = {}
    t["xT_tok"] = nc.dram_tensor("xT_tok", [D, TPC], BF16,
                                 kind="ExternalInput")
    t["xe8"] = nc.dram_tensor("xe8", [EPC, NKP, NPASS, P, 2, HALF], F8,
                              kind="ExternalInput")
    t["sw1"] = nc.dram_tensor("sw1", [S_EXP, D, HS], BF16, kind="ExternalInput")
    t["sw2"] = nc.dram_tensor("sw2", [S_EXP, HS, D], BF16, kind="ExternalInput")
    t["ew1q"] = nc.dram_tensor("ew1q", [EPC, NKP, P, 2, HR], F8,
                               kind="ExternalInput")
    t["ew2q"] = nc.dram_tensor("ew2q", [EPC, RKP, P, 2, D], F8,
                               kind="ExternalInput")
    t["scaleb"] = nc.dram_tensor("scaleb", [EPC, NPASS, P, HALF], BF16,
                                 kind="ExternalInput")
    t["sb1T"] = nc.dram_tensor("sb1T", [P, S_EXP * HS // P], F32,
                               kind="ExternalInput")
    t["eb1T"] = nc.dram_tensor("eb1T", [P, EPC * HR // P], F32,
                               kind="ExternalInput")
    t["ysh"] = nc.dram_tensor("ysh", [TPC, D], F32, kind="ExternalOutput")
    t["yrt"] = nc.dram_tensor("yrt", [EPC, CAP, D], F32, kind="ExternalOutput")

    with tile.TileContext(nc) as tc, ExitStack() as ctx:
        _emit(nc, tc, ctx, t)
    nc.compile()
    _CACHE["nc"] = nc
    return nc


def _install_profile_hook():
    """Make run_bass_kernel_spmd(trace=True) work in this image (the antenv
    package lacks axon_hooks; provide it and register the ctypes hook)."""
    try:
        from antenv import axon_hooks  # noqa: F401
        return
    except ImportError:
        pass
    import antenv
    mod = types.ModuleType("antenv.axon_hooks")
    _hook = [None]
    mod.set_axon_ntff_profile_hook = lambda h: _hook.__setitem__(0, h)
    mod.get_axon_ntff_profile_hook = lambda: _hook[0]
    sys.modules["antenv.axon_hooks"] = mod
    antenv.axon_hooks = mod
    try:
        from trn_agent_boot.trn_boot import _ntff_profile_via_ctypes
        hook = _ntff_profile_via_ctypes("/opt/axon/libaxon_pjrt.so")
        if hook is not None:
            mod.set_axon_ntff_profile_hook(hook)
    except Exception:
        pass


def _gelu_np(x):
    from scipy.special import erf
    return 0.5 * x * (1.0 + erf(x / np.sqrt(2.0)))


def kernel(x, gate_w, gate_b, ew1, eb1, ew2, eb2, sw1, sb1, sw2, sb2,
           _trace=False, _trace_cores=None):
    x = np.asarray(x, np.float32)
    gate_w = np.asarray(gate_w, np.float32)
    gate_b = np.asarray(gate_b, np.float32)
    ew1 = np.asarray(ew1, np.float32)
    eb1 = np.asarray(eb1, np.float32)
    ew2 = np.asarray(ew2, np.float32)
    eb2 = np.asarray(eb2, np.float32)
    sw1 = np.asarray(sw1, np.float32)
    sb1 = np.asarray(sb1, np.float32)
    sw2 = np.asarray(sw2, np.float32)
    sb2 = np.asarray(sb2, np.float32)

    b, s, d = x.shape
    assert b * s == N and d == D, (x.shape, "kernel hardcodes [4,2048,2048]")
    xf = np.ascontiguousarray(x.reshape(-1, d))

    # ---- routing on host (this *is* the dispatch/sharding step) ----
    logits = xf @ gate_w + gate_b
    logits -= logits.max(axis=-1, keepdims=True)
    g = np.exp(logits, dtype=np.float32)
    g /= g.sum(axis=-1, keepdims=True)
    topi = np.argpartition(-g, TOPK, axis=1)[:, :TOPK]          # [N, 2]
    topv = np.take_along_axis(g, topi, axis=1)                  # [N, 2]

    flat_e = topi.ravel()                                       # pair p = 2n+k
    flat_w = topv.ravel()
    flat_tok = np.repeat(np.arange(N, dtype=np.int64), TOPK)
    order = np.argsort(flat_e, kind="stable")
    counts = np.bincount(flat_e, minlength=E)
    starts = np.concatenate([[0], np.cumsum(counts)[:-1]])
    ranks = np.empty(N * TOPK, np.int64)
    ranks[order] = np.arange(N * TOPK) - starts[flat_e[order]]
    ok = ranks < CAP

    # pack tokens per expert in fp8 (D-major), padding slots -> zero column
    xfb = xf.astype(bf16_np)
    xT_all = np.ascontiguousarray(xfb.T)                        # [D, N] bf16
    x8 = xf.astype(f8_np)                                       # [N, D] fp8
    x8T_pad = np.concatenate(
        [np.ascontiguousarray(x8.T), np.zeros((D, 1), f8_np)], axis=1)
    xe_idx = np.full((E, CAP), N, np.int64)
    xe_idx[flat_e[ok], ranks[ok]] = flat_tok[ok]
    # [D, E, CAP] fp8 gathered token columns
    xe8_all = x8T_pad[:, xe_idx.reshape(-1)].reshape(D, E, CAP)

    sc_all = np.zeros((E, CAP), np.float32)
    sc_all[flat_e[ok], ranks[ok]] = flat_w[ok]

    sw1b = sw1.astype(bf16_np)
    sw2b = sw2.astype(bf16_np)
    # routed weights: fp8 with pow2 scale, DoubleRow k-pair interleaved
    ew1q_all = (ew1 * S_E1).astype(f8_np)                       # [E, D, HR]
    ew2q_all = (ew2 * S_E2).astype(f8_np)                       # [E, HR, D]
    # [E, D(kp,i,p), HR] -> [E, NKP, P, 2, HR]
    ew1q_all = np.ascontiguousarray(
        ew1q_all.reshape(E, NKP, 2, P, HR).transpose(0, 1, 3, 2, 4))
    # [E, HR(kp,i,p), D] -> [E, RKP, P, 2, D]
    ew2q_all = np.ascontiguousarray(
        ew2q_all.reshape(E, RKP, 2, P, D).transpose(0, 1, 3, 2, 4))
    sb1T = np.ascontiguousarray(
        sb1.reshape(S_EXP * HS // P, P).T).astype(np.float32)
    sb2_sum = sb2.sum(axis=0).astype(np.float32)

    _install_neff_cache()
    nc = _get_compiled()
    if _trace:
        _install_profile_hook()

    in_maps = []
    for c in range(NCORES):
        el, eh = c * EPC, (c + 1) * EPC
        eb1T = np.ascontiguousarray(
            eb1[el:eh].reshape(EPC * HR // P, P).T).astype(np.float32)
        # xe8: [D, EPC, CAP] -> [EPC, NKP, NPASS, P, 2, HALF]
        xe_c = xe8_all[:, el:eh, :]                             # [D, 2, CAP]
        xe_c = xe_c.reshape(NKP, 2, P, EPC, NPASS, HALF)
        xe_c = np.ascontiguousarray(xe_c.transpose(3, 0, 4, 2, 1, 5))
        sc_c = np.broadcast_to(
            sc_all[el:eh].reshape(EPC, NPASS, 1, HALF),
            (EPC, NPASS, P, HALF))
        in_maps.append({
            "xT_tok": np.ascontiguousarray(xT_all[:, c * TPC:(c + 1) * TPC]),
            "xe8": xe_c,
            "sw1": sw1b,
            "sw2": sw2b,
            "ew1q": np.ascontiguousarray(ew1q_all[el:eh]),
            "ew2q": np.ascontiguousarray(ew2q_all[el:eh]),
            "scaleb": np.ascontiguousarray(sc_c).astype(bf16_np),
            "sb1T": sb1T,
            "eb1T": eb1T,
        })

    if _trace and _trace_cores is None:
        _trace_cores = list(range(NCORES))
    res = run_bass_kernel_spmd(
        nc, in_maps, core_ids=list(range(NCORES)),
        trace=_trace, trace_cores=_trace_cores if _trace else None)
    kernel.last_results = res

    # ---- assemble ----
    out = np.empty((N, D), np.float32)
    for c in range(NCORES):
        out[c * TPC:(c + 1) * TPC] = res.results[c]["ysh"] + sb2_sum

    yrt_all = np.empty((E, CAP, D), np.float32)
    for c in range(NCORES):
        yrt_all[c * EPC:(c + 1) * EPC] = res.results[c]["yrt"]
    yrt_all *= (1.0 / S_E2)                                     # fp8 descale
    flat_rows = yrt_all.reshape(E * CAP, D)
    for k in range(TOPK):
        pk = np.arange(N) * TOPK + k
        okk = ok[pk]
        pos = flat_e[pk] * CAP + ranks[pk]
        if okk.all():
            out += flat_rows[pos]
        else:
            out[okk] += flat_rows[pos[okk]]
            # exact host fallback for overflow assignments, batched per expert
            bad = np.nonzero(~okk)[0]
            for e_ in np.unique(flat_e[pk[bad]]):
                sel = bad[flat_e[pk[bad]] == e_]
                h_ = _gelu_np(xf[sel] @ ew1[e_] + eb1[e_])
                out[sel] += flat_w[pk[sel], None] * (h_ @ ew2[e_] + eb2[e_])

    if np.any(eb2):
        for k in range(TOPK):
            out += topv[:, k:k + 1] * eb2[topi[:, k]]

    return out.reshape(b, s, d)


# revision 14
# speedup vs baseline: 1.2385x; 1.0029x over previous
"""DeepSeek-style MoE (top-2 of 16 routed experts + 2 dense shared experts)
on 8 Trainium2 NeuronCores.

Sharding (hardcoded for x:[4,2048,2048], D=2048, E=16, H_R=512, H_S=8192):
  - Gate (softmax + top-2) on host as part of dispatch; tokens packed per
    expert host-side while building per-core shards.
  - Shared experts: data-parallel, 1024 tokens/core, bf16 matmuls, weights
    streamed through SBUF once. L1->L2 software-pipelined ACROSS chunks so
    the PE never waits on the gelu at a chunk boundary.
  - Routed experts: expert-parallel, 2 experts/core, capacity 1152 slots
    (max observed load 1087), computed ENTIRELY in fp8(e4m3) with
    MatmulPerfMode.DoubleRow (2x bf16 MAC rate; contributes ~0.3% rel err
    because routed RMS is ~20x below shared RMS). Weights carry a pow2
    scale (folded out via the gelu activation scale on L1 and a host-side
    descale of yrt for L2). Combine weights applied on-device; overflow
    slots fall back to exact host computation.
"""
import sys
import types
from contextlib import ExitStack

import numpy as np

_TRN = "/opt/trn_rl_repo"
if _TRN not in sys.path:
    sys.path.insert(0, _TRN)

import ml_dtypes  # noqa: E402
import concourse.mybir as mybir  # noqa: E402
import concourse.tile as tile  # noqa: E402
from concourse import bacc  # noqa: E402
from concourse.bass_utils import run_bass_kernel_spmd  # noqa: E402

BF16 = mybir.dt.bfloat16
F8 = mybir.dt.float8e4
F32 = mybir.dt.float32
GELU = mybir.ActivationFunctionType.Gelu
ADD = mybir.AluOpType.add
MULT = mybir.AluOpType.mult
DR = mybir.MatmulPerfMode.DoubleRow
bf16_np = ml_dtypes.bfloat16
f8_np = ml_dtypes.float8_e4m3

P = 128
D = 2048          # model dim
E = 16            # routed experts
TOPK = 2
HS = 8192         # shared-expert hidden
HR = 512          # routed-expert hidden
S_EXP = 2         # shared experts
NCORES = 8
N = 8192          # tokens
TPC = N // NCORES     # tokens per core (1024)
EPC = E // NCORES     # routed experts per core (2)
CAP = 1152            # routed capacity per expert (max seen load 1087)
HALF = 384            # routed tokens per pass
NPASS = CAP // HALF   # 3
NKD = D // P          # 16 contraction tiles over D (bf16 shared path)
NKP = D // (2 * P)    # 8 k-tile PAIRS over D (fp8 DoubleRow routed path)
RKP = HR // (2 * P)   # 2 hidden k-tile pairs (routed L2)
CH = 512              # shared-expert H chunk
NCH = HS // CH        # 16 chunks per shared expert
NT = TPC // P         # 8 token tiles per core
NDC = D // 512        # 4 output-D chunks

S_E1 = 1024.0         # pow2 scale baked into routed fp8 weights (W*S fits e4m3)
S_E2 = 1024.0


def _emit(nc, tc, ctx, t):
    """Emit the tile program. `t` is the dict of DRAM tensor handles."""
    xacts = ctx.enter_context(tc.tile_pool(name="xacts", bufs=32))
    wslab = ctx.enter_context(tc.tile_pool(name="wslab", bufs=32))
    xepool = ctx.enter_context(tc.tile_pool(name="xepool", bufs=24))
    w2slab = ctx.enter_context(tc.tile_pool(name="w2slab", bufs=4))
    e2pool = ctx.enter_context(tc.tile_pool(name="e2pool", bufs=4))
    hpool = ctx.enter_context(tc.tile_pool(name="hpool", bufs=11))
    hqpool = ctx.enter_context(tc.tile_pool(name="hqpool", bufs=6))
    scpool = ctx.enter_context(tc.tile_pool(name="scpool", bufs=3))
    ypool = ctx.enter_context(tc.tile_pool(name="ypool", bufs=8))
    cpool = ctx.enter_context(tc.tile_pool(name="cpool", bufs=1))
    psA = ctx.enter_context(tc.tile_pool(name="psA", bufs=2, space="PSUM"))
    psB = ctx.enter_context(tc.tile_pool(name="psB", bufs=3, space="PSUM"))

    # constants
    sb1T = cpool.tile([P, S_EXP * HS // P], F32, name="sb1T")       # [128, 128]
    nc.sync.dma_start(sb1T[:], t["sb1T"][:, :])
    eb1T = cpool.tile([P, EPC * HR // P], F32, name="eb1T")         # [128, 8]
    nc.sync.dma_start(eb1T[:], t["eb1T"][:, :])

    # x^T resident, split in 512-token halves so the first L1 chain can start
    # as soon as the first ~400KB lands (PE starts ~2us in, not ~12us).
    xTn = [[], []]
    w1s_first = []
    for k in range(NKD):
        xt = xacts.tile([P, 512], BF16, name="xT0", tag="xacts")
        nc.sync.dma_start(xt[:], t["xT_tok"][k * P:(k + 1) * P, 0:512])
        xTn[0].append(xt)
        w = wslab.tile([P, CH], BF16, name="w1s", tag="wslab")
        nc.sync.dma_start(w[:], t["sw1"][0, k * P:(k + 1) * P, 0:CH])
        w1s_first.append(w)
    for k in range(NKD):
        xt = xacts.tile([P, 512], BF16, name="xT1", tag="xacts")
        nc.sync.dma_start(xt[:], t["xT_tok"][k * P:(k + 1) * P, 512:TPC])
        xTn[1].append(xt)

    y_tiles = [None] * NT

    # ---- shared experts: y[tok, D] += sum_s W2_s^T gelu(W1_s^T x^T + b1) ----
    # A(s,c): hT chunk [CH, TPC] = gelu(W1[:, chunk]^T @ xT + b1)
    # B(s,c): y += hT-contracted W2 chunk.  Emitted pipelined: A(next) before
    # B(cur) so the PE fills the gelu latency window with A's matmuls.
    def emit_sh_A(s, c):
        if s == 0 and c == 0:
            w1s = w1s_first
        else:
            w1s = []
            for k in range(NKD):
                w = wslab.tile([P, CH], BF16, name="w1s", tag="wslab")
                nc.sync.dma_start(
                    w[:],
                    t["sw1"][s, k * P:(k + 1) * P, c * CH:(c + 1) * CH])
                w1s.append(w)
        hts = []
        for h in range(CH // P):
            ps = psA.tile([P, TPC], F32, name="psA", tag="psA")
            for n in range(TPC // 512):
                for k in range(NKD):
                    nc.tensor.matmul(
                        ps[:, n * 512:(n + 1) * 512],
                        w1s[k][:, h * P:(h + 1) * P],
                        xTn[n][k][:, :],
                        start=(k == 0), stop=(k == NKD - 1))
            ht = hpool.tile([P, TPC], BF16, name="ht", tag="hpool")
            nc.scalar.activation(ht[:], ps[:], GELU,
                                 bias=sb1T[:, s * 64 + c * 4 + h:
                                           s * 64 + c * 4 + h + 1])
            hts.append(ht)
        return hts

    def emit_sh_B(s, c, hts):
        first = (s == 0 and c == 0)
        w2s = []
        for kh in range(CH // P):
            w = w2slab.tile([P, D], BF16, name="w2s", tag="w2slab")
            nc.sync.dma_start(
                w[:], t["sw2"][s, c * CH + kh * P:c * CH + (kh + 1) * P, :])
            w2s.append(w)
        for ti in range(NT):
            for n in range(NDC):
                ps = psB.tile([P, 512], F32, name="psB", tag="psB")
                for kh in range(CH // P):
                    nc.tensor.matmul(
                        ps[:, :],
                        hts[kh][:, ti * P:(ti + 1) * P],
                        w2s[kh][:, n * 512:(n + 1) * 512],
                        start=(kh == 0), stop=(kh == CH // P - 1))
                if first:
                    if n == 0:
                        y_tiles[ti] = ypool.tile([P, D], F32, name="y",
                                                 tag="ypool")
                    nc.vector.tensor_copy(
                        y_tiles[ti][:, n * 512:(n + 1) * 512], ps[:, :])
                else:
                    nc.vector.tensor_tensor(
                        y_tiles[ti][:, n * 512:(n + 1) * 512],
                        y_tiles[ti][:, n * 512:(n + 1) * 512],
                        ps[:, :], op=ADD)

    # ---- routed experts: fp8 DoubleRow, 2 experts/core, NPASS passes ----
    expert_e1s = {}
    expert_e2s = {}
    pass_in = {}

    def load_e1s(e):
        e1s = []
        for kp in range(NKP):
            w = wslab.tile([P, 2, HR], F8, name="e1s", tag="wslab")
            nc.sync.dma_start(w[:], t["ew1q"][e, kp, :, :, :])
            e1s.append(w)
        expert_e1s[e] = e1s

    def load_e2s(e):
        e2s = []
        for kp in range(RKP):
            w = e2pool.tile([P, 2, D], F8, name="e2s", tag="e2pool")
            nc.sync.dma_start(w[:], t["ew2q"][e, kp, :, :, :])
            e2s.append(w)
        expert_e2s[e] = e2s

    def load_pass(e, half):
        sc = scpool.tile([P, HALF], BF16, name="sc", tag="scpool")
        nc.sync.dma_start(sc[:], t["scaleb"][e, half, :, :])
        xe = []
        for kp in range(NKP):
            xt = xepool.tile([P, 2, HALF], F8, name="xe", tag="xepool")
            nc.sync.dma_start(xt[:], t["xe8"][e, kp, half, :, :, :])
            xe.append(xt)
        pass_in[(e, half)] = (sc, xe)

    sh_pending = None
    for s in range(S_EXP):
        for c in range(NCH):
            hts = emit_sh_A(s, c)
            if (s, c) == (S_EXP - 1, NCH - 2):
                # issue routed input DMAs ahead of the last shared chunk so
                # they don't queue behind its 4MB of weight slabs
                load_e1s(0)
                load_e2s(0)
                load_pass(0, 0)
                load_pass(0, 1)
            if sh_pending is not None:
                emit_sh_B(*sh_pending)
            sh_pending = (s, c, hts)
    # last shared B is emitted interleaved with the first routed A below.

    def emit_A(e, half):
        e1s = expert_e1s[e]
        sc, xe = pass_in.pop((e, half))
        hqs = [hqpool.tile([P, 2, HALF], F8, name="hq", tag="hqpool")
               for _ in range(RKP)]
        for h in range(HR // P):
            ps = psA.tile([P, HALF], F32, name="psAr", tag="psA")
            for kp in range(NKP):
                nc.tensor.matmul(
                    ps[:, :],
                    e1s[kp][:, :, h * P:(h + 1) * P],
                    xe[kp][:, :, :],
                    start=(kp == 0), stop=(kp == NKP - 1),
                    perf_mode=DR)
            ht = hpool.tile([P, HALF], BF16, name="htr", tag="hpool")
            nc.scalar.activation(ht[:], ps[:], GELU,
                                 bias=eb1T[:, e * 4 + h:e * 4 + h + 1],
                                 scale=1.0 / S_E1)
            nc.vector.tensor_tensor(hqs[h // 2][:, h % 2, :],
                                    ht[:], sc[:], op=MULT)
        return hqs

    def emit_B(e, half, hqs):
        e2s = expert_e2s[e]
        for ti in range(HALF // P):
            st = ypool.tile([P, D], F32, name="str", tag="ypool")
            for n in range(NDC):
                ps = psB.tile([P, 512], F32, name="psBr", tag="psB")
                for kp in range(RKP):
                    nc.tensor.matmul(
                        ps[:, :],
                        hqs[kp][:, :, ti * P:(ti + 1) * P],
                        e2s[kp][:, :, n * 512:(n + 1) * 512],
                        start=(kp == 0), stop=(kp == RKP - 1),
                        perf_mode=DR)
                nc.vector.tensor_copy(st[:, n * 512:(n + 1) * 512], ps[:, :])
            nc.sync.dma_start(
                t["yrt"][e, half * HALF + ti * P:
                         half * HALF + (ti + 1) * P, :], st[:])

    passes = [(e, h) for e in range(EPC) for h in range(NPASS)]
    pend = []
    for i, (e, half) in enumerate(passes):
        if i + 2 < len(passes):
            load_pass(*passes[i + 2])
        if e + 1 < EPC and half == NPASS - 2:
            load_e1s(e + 1)
        if e + 1 < EPC and half == NPASS - 1:
            load_e2s(e + 1)
        hqs = emit_A(e, half)
        if sh_pending is not None:
            # last shared-expert B slots in behind the first routed A
            emit_sh_B(*sh_pending)
            sh_pending = None
            for ti in range(NT):
                nc.sync.dma_start(t["ysh"][ti * P:(ti + 1) * P, :],
                                  y_tiles[ti][:])
        if len(pend) >= 2:
            emit_B(*pend.pop(0))
        pend.append((e, half, hqs))
    for p in pend:
        emit_B(*p)


def _install_neff_cache():
    """Disk-cache walrus NEFF compiles keyed by BIR hash (compile is ~5min)."""
    import concourse.bass2jax as b2j
    if getattr(b2j, "_neff_cache_installed", False):
        return
    import hashlib
    import os
    import shutil
    orig = b2j.compile_bir_kernel
    cache_dir = "/tmp/bass_neff_cache"

    def cached(bir_json, tmpdir, neff_name="file.neff"):
        try:
            os.makedirs(cache_dir, exist_ok=True)
            h = hashlib.sha256(bir_json).hexdigest()[:24]
            cpath = os.path.join(cache_dir, h + ".neff")
            if os.path.exists(cpath):
                dst = os.path.join(tmpdir, neff_name)
                shutil.copy(cpath, dst)
                return dst
            p = orig(bir_json, tmpdir, neff_name)
            shutil.copy(p, cpath)
            return p
        except OSError:
            return orig(bir_json, tmpdir, neff_name)

    b2j.compile_bir_kernel = cached
    b2j._neff_cache_installed = True


_CACHE = {}


def _get_compiled():
    if "nc" in _CACHE:
        return _CACHE["nc"]
    nc = bacc.Bacc("TRN2", target_bir_lowering=False, debug=False,
                   num_devices=NCORES)
    t = {}
    t["xT_tok"] = nc.dram_tensor("xT_tok", [D, TPC], BF16,
                                 kind="ExternalInput")
    t["xe8"] = nc.dram_tensor("xe8", [EPC, NKP, NPASS, P, 2, HALF], F8,
                              kind="ExternalInput")
    t["sw1"] = nc.dram_tensor("sw1", [S_EXP, D, HS], BF16, kind="ExternalInput")
    t["sw2"] = nc.dram_tensor("sw2", [S_EXP, HS, D], BF16, kind="ExternalInput")
    t["ew1q"] = nc.dram_tensor("ew1q", [EPC, NKP, P, 2, HR], F8,
                               kind="ExternalInput")
    t["ew2q"] = nc.dram_tensor("ew2q", [EPC, RKP, P, 2, D], F8,
                               kind="ExternalInput")
    t["scaleb"] = nc.dram_tensor("scaleb", [EPC, NPASS, P, HALF], BF16,
                                 kind="ExternalInput")
    t["sb1T"] = nc.dram_tensor("sb1T", [P, S_EXP * HS // P], F32,
                               kind="ExternalInput")
    t["eb1T"] = nc.dram_tensor("eb1T", [P, EPC * HR // P], F32,
                               kind="ExternalInput")
    t["ysh"] = nc.dram_tensor("ysh", [TPC, D], F32, kind="ExternalOutput")
    t["yrt"] = nc.dram_tensor("yrt", [EPC, CAP, D], F32, kind="ExternalOutput")

    with tile.TileContext(nc) as tc, ExitStack() as ctx:
        _emit(nc, tc, ctx, t)
    nc.compile()
    _CACHE["nc"] = nc
    return nc


def _install_profile_hook():
    """Make run_bass_kernel_spmd(trace=True) work in this image (the antenv
    package lacks axon_hooks; provide it and register the ctypes hook)."""
    try:
        from antenv import axon_hooks  # noqa: F401
        return
    except ImportError:
        pass
    import antenv
    mod = types.ModuleType("antenv.axon_hooks")
    _hook = [None]
    mod.set_axon_ntff_profile_hook = lambda h: _hook.__setitem__(0, h)
    mod.get_axon_ntff_profile_hook = lambda: _hook[0]
    sys.modules["antenv.axon_hooks"] = mod
    antenv.axon_hooks = mod
    try:
        from trn_agent_boot.trn_boot import _ntff_profile_via_ctypes
        hook = _ntff_profile_via_ctypes("/opt/axon/libaxon_pjrt.so")
        if hook is not None:
            mod.set_axon_ntff_profile_hook(hook)
    except Exception:
        pass


def _gelu_np(x):
    from scipy.special import erf
    return 0.5 * x * (1.0 + erf(x / np.sqrt(2.0)))


def kernel(x, gate_w, gate_b, ew1, eb1, ew2, eb2, sw1, sb1, sw2, sb2,
           _trace=False, _trace_cores=None):
    x = np.asarray(x, np.float32)
    gate_w = np.asarray(gate_w, np.float32)
    gate_b = np.asarray(gate_b, np.float32)
    ew1 = np.asarray(ew1, np.float32)
    eb1 = np.asarray(eb1, np.float32)
    ew2 = np.asarray(ew2, np.float32)
    eb2 = np.asarray(eb2, np.float32)
    sw1 = np.asarray(sw1, np.float32)
    sb1 = np.asarray(sb1, np.float32)
    sw2 = np.asarray(sw2, np.float32)
    sb2 = np.asarray(sb2, np.float32)

    b, s, d = x.shape
    assert b * s == N and d == D, (x.shape, "kernel hardcodes [4,2048,2048]")
    xf = np.ascontiguousarray(x.reshape(-1, d))

    # ---- routing on host (this *is* the dispatch/sharding step) ----
    logits = xf @ gate_w + gate_b
    logits -= logits.max(axis=-1, keepdims=True)
    g = np.exp(logits, dtype=np.float32)
    g /= g.sum(axis=-1, keepdims=True)
    topi = np.argpartition(-g, TOPK, axis=1)[:, :TOPK]          # [N, 2]
    topv = np.take_along_axis(g, topi, axis=1)                  # [N, 2]

    flat_e = topi.ravel()                                       # pair p = 2n+k
    flat_w = topv.ravel()
    flat_tok = np.repeat(np.arange(N, dtype=np.int64), TOPK)
    order = np.argsort(flat_e, kind="stable")
    counts = np.bincount(flat_e, minlength=E)
    starts = np.concatenate([[0], np.cumsum(counts)[:-1]])
    ranks = np.empty(N * TOPK, np.int64)
    ranks[order] = np.arange(N * TOPK) - starts[flat_e[order]]
    ok = ranks < CAP

    # pack tokens per expert in fp8 (D-major), padding slots -> zero column
    xfb = xf.astype(bf16_np)
    xT_all = np.ascontiguousarray(xfb.T)                        # [D, N] bf16
    x8 = xf.astype(f8_np)                                       # [N, D] fp8
    x8T_pad = np.concatenate(
        [np.ascontiguousarray(x8.T), np.zeros((D, 1), f8_np)], axis=1)
    xe_idx = np.full((E, CAP), N, np.int64)
    xe_idx[flat_e[ok], ranks[ok]] = flat_tok[ok]
    # [D, E, CAP] fp8 gathered token columns
    xe8_all = x8T_pad[:, xe_idx.reshape(-1)].reshape(D, E, CAP)

    sc_all = np.zeros((E, CAP), np.float32)
    sc_all[flat_e[ok], ranks[ok]] = flat_w[ok]

    sw1b = sw1.astype(bf16_np)
    sw2b = sw2.astype(bf16_np)
    # routed weights: fp8 with pow2 scale, DoubleRow k-pair interleaved
    ew1q_all = (ew1 * S_E1).astype(f8_np)                       # [E, D, HR]
    ew2q_all = (ew2 * S_E2).astype(f8_np)                       # [E, HR, D]
    # [E, D(kp,i,p), HR] -> [E, NKP, P, 2, HR]
    ew1q_all = np.ascontiguousarray(
        ew1q_all.reshape(E, NKP, 2, P, HR).transpose(0, 1, 3, 2, 4))
    # [E, HR(kp,i,p), D] -> [E, RKP, P, 2, D]
    ew2q_all = np.ascontiguousarray(
        ew2q_all.reshape(E, RKP, 2, P, D).transpose(0, 1, 3, 2, 4))
    sb1T = np.ascontiguousarray(
        sb1.reshape(S_EXP * HS // P, P).T).astype(np.float32)
    sb2_sum = sb2.sum(axis=0).astype(np.float32)

    _install_neff_cache()
    nc = _get_compiled()
    if _trace:
        _install_profile_hook()

    in_maps = []
    for c in range(NCORES):
        el, eh = c * EPC, (c + 1) * EPC
        eb1T = np.ascontiguousarray(
            eb1[el:eh].reshape(EPC * HR // P, P).T).astype(np.float32)
        # xe8: [D, EPC, CAP] -> [EPC, NKP, NPASS, P, 2, HALF]
        xe_c = xe8_all[:, el:eh, :]                             # [D, 2, CAP]
        xe_c = xe_c.reshape(NKP, 2, P, EPC, NPASS, HALF)
        xe_c = np.ascontiguousarray(xe_c.transpose(3, 0, 4, 2, 1, 5))
        sc_c = np.broadcast_to(
            sc_all[el:eh].reshape(EPC, NPASS, 1, HALF),
            (EPC, NPASS, P, HALF))
        in_maps.append({
            "xT_tok": np.ascontiguousarray(xT_all[:, c * TPC:(c + 1) * TPC]),
            "xe8": xe_c,
            "sw1": sw1b,
            "sw2": sw2b,
            "ew1q": np.ascontiguousarray(ew1q_all[el:eh]),
            "ew2q": np.ascontiguousarray(ew2q_all[el:eh]),
            "scaleb": np.ascontiguousarray(sc_c).astype(bf16_np),
            "sb1T": sb1T,
            "eb1T": eb1T,
        })

    if _trace and _trace_cores is None:
        _trace_cores = list(range(NCORES))
    res = run_bass_kernel_spmd(
        nc, in_maps, core_ids=list(range(NCORES)),
        trace=_trace, trace_cores=_trace_cores if _trace else None)
    kernel.last_results = res

    # ---- assemble ----
    out = np.empty((N, D), np.float32)
    for c in range(NCORES):
        out[c * TPC:(c + 1) * TPC] = res.results[c]["ysh"] + sb2_sum

    yrt_all = np.empty((E, CAP, D), np.float32)
    for c in range(NCORES):
        yrt_all[c * EPC:(c + 1) * EPC] = res.results[c]["yrt"]
    yrt_all *= (1.0 / S_E2)                                     # fp8 descale
    flat_rows = yrt_all.reshape(E * CAP, D)
    for k in range(TOPK):
        pk = np.arange(N) * TOPK + k
        okk = ok[pk]
        pos = flat_e[pk] * CAP + ranks[pk]
        if okk.all():
            out += flat_rows[pos]
        else:
            out[okk] += flat_rows[pos[okk]]
            # exact host fallback for overflow assignments, batched per expert
            bad = np.nonzero(~okk)[0]
            for e_ in np.unique(flat_e[pk[bad]]):
                sel = bad[flat_e[pk[bad]] == e_]
                h_ = _gelu_np(xf[sel] @ ew1[e_] + eb1[e_])
                out[sel] += flat_w[pk[sel], None] * (h_ @ ew2[e_] + eb2[e_])

    if np.any(eb2):
        for k in range(TOPK):
            out += topv[:, k:k + 1] * eb2[topi[:, k]]

    return out.reshape(b, s, d)
